# revision 39
# baseline (speedup 1.0000x reference)
"""Trainium2 Bass kernel for nn_CrossAttentionBlock (B=2, N=2048, C=1024, H=16).

Sharding: 8 cores, zero collectives. Cores 0-3 own batch 0, cores 4-7 batch 1;
core r in its group owns queries [512r, 512r+512). Each core projects the FULL
batch K and V locally (replicated within the group) so attention needs no
AllGather; Q/out-proj/FFN are computed only for the core's own 512 tokens.

Compute layout (fp8 e4m3 matmuls with DoubleRow = 2 contraction-tiles/pass):
  khT  [hd, kt]   = Wk^T @ kT        (head pair m lives at partitions 0..127)
  qhT  [hd, q]    = Wq^T @ qT
  vhx  [kt, h, 65]= vT^T @ Wv natural, with a ones column so the ctx matmul
                    also produces sum(exp) in row 64
  S^T  [kt, q]    = khT_h^T @ qhT_h   (64-contraction, fp8)
  exp             ACT engine (exact) for some kt tiles, DVE Schraudolph
                    (exp2 via int bit-trick on the fp8/bf16 representation)
                    for the rest; constant bias cancels in normalization
  ctx^T[65, q]    = vhx^T @ expS^T accumulated over kt in PSUM (DoubleRow)
  out-proj        deferred: all 8 head-pairs accumulate in one PSUM bank
  x    [tok, c]   = psum + (q + bo + bv@Wo)    (bv folded via sum(p)=1)
  LN              (x-mu)*rstd only; ln_w folded into W1, ln_b into b1
  FFN  fp8 DR     a^T = W1'^T @ hT; gelu; y = gT^T @ W2 + x + b2
"""
import sys

sys.path.insert(0, "/opt/trn_rl_repo")

import numpy as np
import ml_dtypes

import concourse.bass as bass
import concourse.tile as tile
from concourse import bacc, mybir
from concourse.bass_utils import run_bass_kernel_spmd
from concourse.masks import make_identity


def _ensure_ntff_hook():
    """The agent image's antenv package lacks axon_hooks; synthesize it so
    run_bass_kernel_spmd(trace=True) can reach the libaxon NTFF profiler."""
    import types
    if "antenv.axon_hooks" in sys.modules:
        return
    try:
        import antenv
    except ImportError:
        return
    mod = types.ModuleType("antenv.axon_hooks")
    mod._hook = None
    mod.set_axon_ntff_profile_hook = lambda h: setattr(mod, "_hook", h)
    mod.get_axon_ntff_profile_hook = lambda: mod._hook
    sys.modules["antenv.axon_hooks"] = mod
    antenv.axon_hooks = mod
    try:
        from trn_agent_boot.trn_boot import _ntff_profile_via_ctypes
        hook = _ntff_profile_via_ctypes("/opt/axon/libaxon_pjrt.so")
        if hook is not None:
            mod._hook = hook
    except Exception:
        pass


_ensure_ntff_hook()

P = 128
NT = 512          # q-tokens per core
KT = 2048         # keys per batch
B, N, C, H, HD, FF = 2, 2048, 1024, 16, 64, 2048
CT = C // P       # 8 c-tiles
CP = CT // 2      # 4 c-tile pairs (DoubleRow contraction granularity)
JT = NT // P      # 4 tok-tiles per core
NKT = KT // P     # 16 kt-tiles
NCK = KT // NT    # 4 kt chunks of 512
SCALE = HD ** -0.5
LOG2E = 1.4426950408889634
# e4m3's min normal is 2^-6 but the weights have std 0.02: scale them x16 on
# the host so they sit in the healthy normal range, and unscale downstream:
# scores carry WS^2 (folded into the exp scale), V carries WS into stack
# (folded with Wo's WS into the out-proj accumulate), FFN1 carries WS (folded
# into gelu's scale arg), FFN2 carries WS (folded into the residual add).
WS = 16.0

F32 = mybir.dt.float32
BF16 = mybir.dt.bfloat16
AF = mybir.ActivationFunctionType
ALU = mybir.AluOpType

FP8 = True                      # fp8 e4m3 + DoubleRow everywhere possible
FP8_FFN = False                  # FFN in fp8 too (flip to bf16 if rel err high)
N_DVE_EXP = 3                   # of 8 kt-megas per parity, how many exp on DVE

if FP8:
    LOWP = mybir.dt.float8e4
    DR = mybir.MatmulPerfMode.DoubleRow
else:
    LOWP = BF16
    DR = None
FFNP = mybir.dt.float8e4 if FP8_FFN else BF16
FFN_DR = mybir.MatmulPerfMode.DoubleRow if FP8_FFN else None
# Raw PSUM scores carry WS^2 from the scaled Wq/Wk; exp must undo it.
ESCALE = SCALE / (WS * WS)
# Schraudolph constants: int8 bits of e4m3 = 8*(log2(x)+7); bf16: 128*(l2+127)
if FP8:
    EXP_A, EXP_B, EXP_IDT = ESCALE * LOG2E * 8.0, 56.0, mybir.dt.int8
else:
    EXP_A, EXP_B, EXP_IDT = ESCALE * LOG2E * 128.0, 16256.0, mybir.dt.int16


def build():
    nc = bacc.Bacc(trn_type="TRN2")

    # ---- DRAM parameters (per-core shards; weights replicated) ----
    # kT/vT: full batch, chunk-major: [ck, p, cp, s, t] = x[b, ck*512+t,
    # (2cp+s)*128+p]; qT: own 512-token slice, same inner layout.
    kT_d = nc.declare_dram_parameter("kT", [P, CP, 2, NT], LOWP, isOutput=False)
    vT_d = nc.declare_dram_parameter("vT", [P, CP, 2, NT], LOWP, isOutput=False)
    qT_d = nc.declare_dram_parameter("qT", [P, CP, 2, NT], LOWP, isOutput=False)
    qres_d = nc.declare_dram_parameter("qres", [NT, C], BF16, isOutput=False)
    Wk_d = nc.declare_dram_parameter("Wk", [P, CP, 2, C], LOWP, isOutput=False)
    Wq_d = nc.declare_dram_parameter("Wq", [P, CP, 2, C], LOWP, isOutput=False)
    Wv_d = nc.declare_dram_parameter("Wv", [P, CP, 2, C], LOWP, isOutput=False)
    Wo_d = nc.declare_dram_parameter("Wo", [P, CP, 2, C], LOWP, isOutput=False)
    W1_d = nc.declare_dram_parameter("W1", [P, CP, 2, FF], FFNP, isOutput=False)
    W2_d = nc.declare_dram_parameter("W2", [P, FF // P // 2, 2, C], FFNP,
                                     isOutput=False)
    bq_d = nc.declare_dram_parameter("bqt", [P, CT], F32, isOutput=False)
    bk_d = nc.declare_dram_parameter("bkt", [P, CT], F32, isOutput=False)
    b1_d = nc.declare_dram_parameter("b1t", [P, FF // P], F32, isOutput=False)
    bres_d = nc.declare_dram_parameter("bresb", [P, C], BF16, isOutput=False)
    b2_d = nc.declare_dram_parameter("b2b", [P, C], BF16, isOutput=False)
    out_d = nc.declare_dram_parameter("out", [NT, C], F32, isOutput=True)

    with tile.TileContext(nc) as tc:
        with (
            tc.tile_pool(name="pers", bufs=1) as pers,
            tc.tile_pool(name="big", bufs=1) as big,
            tc.tile_pool(name="work", bufs=1) as work,
            tc.tile_pool(name="ps", bufs=1, space="PSUM") as ps,
            tc.tile_pool(name="dram", bufs=1, space="DRAM") as dpool,
            tc.tile_pool(name="dsmall", bufs=2, space="DRAM") as dsmall,
        ):
            # ---------------- input / weight loads ----------------
            # sync queue: own kT slice first (K-proj is the critical start),
            # then vT/qT/qres. scalar queue: weights needed early. gpsimd
            # queue: the two AllGathers, then the fat late-phase weights.
            kT_sb = big.tile([P, CP, 2, NT], LOWP)
            nc.sync.dma_start(out=kT_sb[:], in_=kT_d[:])
            Wk_sb = pers.tile([P, CP, 2, C], LOWP)
            nc.scalar.dma_start(out=Wk_sb[:], in_=Wk_d[:])
            bk_sb = pers.tile([P, CT], F32)
            nc.scalar.dma_start(out=bk_sb[:], in_=bk_d[:])
            bq_sb = pers.tile([P, CT], F32)
            nc.scalar.dma_start(out=bq_sb[:], in_=bq_d[:])
            vT_sb = big.tile([P, CP, 2, NT], LOWP)
            nc.sync.dma_start(out=vT_sb[:], in_=vT_d[:])
            qT_sb = big.tile([P, CP, 2, NT], LOWP)
            nc.sync.dma_start(out=qT_sb[:], in_=qT_d[:])
            Wv_sb = pers.tile([P, CP, 2, C], LOWP)
            nc.scalar.dma_start(out=Wv_sb[:], in_=Wv_d[:])
            Wq_sb = pers.tile([P, CP, 2, C], LOWP)
            nc.scalar.dma_start(out=Wq_sb[:], in_=Wq_d[:])
            bres_b = pers.tile([P, C], BF16)
            nc.scalar.dma_start(out=bres_b[:], in_=bres_d[:])
            b1_sb = pers.tile([P, FF // P], F32)
            nc.scalar.dma_start(out=b1_sb[:], in_=b1_d[:])
            ident = pers.tile([P, P], BF16)
            make_identity(nc, ident[:])
            eps_sb = pers.tile([P, 1], F32)
            nc.vector.memset(eps_sb[:], 1e-5)

            # ---------------- persistent activations ----------------
            # tag TA: khT3 (dead after scores) shares its slot with gT3;
            # tag TB: vhx (dead after ctx) shares with hT3.
            khT3 = big.tile([P, CT, KT], LOWP, tag="TA")  # [hd-of-pair, pair, kt]
            qhT3 = big.tile([P, CT, NT], LOWP)           # [hd-of-pair, pair, q]
            vhx = big.tile([P, NKT // 2, 2, H, HD + 1], LOWP, tag="TB")
            x_acc = big.tile([P, JT, C], F32)            # residual accumulator
            stack = big.tile([P, CP, 2, NT], LOWP)       # ctx^T for out-proj

            # =========== PHASE A: projections (no collective) ===========
            # Two m-tiles share one 2-bank PSUM tile with the matmuls issued
            # interleaved across the banks, so the per-instruction PE latency
            # overlaps the other bank's stream.
            def proj_pair(W_sb, mov, mh, nm):
                pp = ps.tile([P, 2, NT], F32, tag="s", bufs=3, name=nm)
                for cp in range(CP):
                    for half in range(2):
                        m = 2 * mh + half
                        if DR:
                            nc.tensor.matmul(pp[:, half, :],
                                             W_sb[:, cp, :, m * P:(m + 1) * P],
                                             mov[:, cp, :, :], perf_mode=DR,
                                             start=(cp == 0), stop=(cp == CP - 1))
                        else:
                            for s in range(2):
                                nc.tensor.matmul(pp[:, half, :],
                                                 W_sb[:, cp, s, m * P:(m + 1) * P],
                                                 mov[:, cp, s, :],
                                                 start=(cp == 0 and s == 0),
                                                 stop=(cp == CP - 1 and s == 1))
                return pp

            # Staging / AllGather buffers: each core projects only its own
            # 512-token K/V slice; an 8-core Shared-output AllGather
            # distributes them and each core extracts its batch group's 4
            # slices via a partition-id-derived dynamic offset.
            khT_own = big.tile([P, CT, NT], LOWP)
            vhx_own = big.tile([P, 2, 2, H, HD + 1], LOWP)
            kag_k_in = dpool.tile([C, NT], LOWP)
            kag_k_out = dpool.tile([8 * C, NT], LOWP, addr_space="Shared")
            kag_v_in = dpool.tile([P, 2, 2, H, HD + 1], LOWP)
            kag_v_out = dpool.tile([8 * P, 2, 2, H, HD + 1], LOWP,
                                   addr_space="Shared")
            RG8 = [[0, 1, 2, 3, 4, 5, 6, 7]]

            # K-proj own slice: khT_own[:, m, :] = sum_cp Wk[:,cp].T @ kT[:,cp]
            for mh in range(CT // 2):
                pk = proj_pair(Wk_sb, kT_sb, mh, f"pk{mh}")
                # bias + fp8 convert on DVE (ACT is reserved for exp later)
                for half in range(2):
                    m = 2 * mh + half
                    nc.vector.tensor_scalar_add(
                        out=khT_own[:, m, :],
                        in0=pk[:, half, :], scalar1=bk_sb[:, m:m + 1])
            nc.scalar.dma_start(
                out=kag_k_in[:].rearrange("(m p) f -> p m f", p=P),
                in_=khT_own[:])
            nc.gpsimd.collective_compute(
                "AllGather", mybir.AluOpType.bypass,
                ins=[kag_k_in[:]], outs=[kag_k_out[:]], replica_groups=RG8)

            # V-proj own slice, natural layout with the ones column baked in
            nc.vector.memset(vhx_own[:, :, :, :, HD:HD + 1], 1.0)
            for il in range(JT):
                pv = ps.tile([P, 2, NT], F32, tag="s", bufs=3, name=f"pv{il}")
                for cp in range(CP):
                    for n in range(2):
                        if DR:
                            nc.tensor.matmul(
                                pv[:, n, :],
                                vT_sb[:, cp, :, il * P:(il + 1) * P],
                                Wv_sb[:, cp, :, n * NT:(n + 1) * NT], perf_mode=DR,
                                start=(cp == 0), stop=(cp == CP - 1))
                        else:
                            for s in range(2):
                                nc.tensor.matmul(
                                    pv[:, n, :],
                                    vT_sb[:, cp, s, il * P:(il + 1) * P],
                                    Wv_sb[:, cp, s, n * NT:(n + 1) * NT],
                                    start=(cp == 0 and s == 0),
                                    stop=(cp == CP - 1 and s == 1))
                nc.vector.tensor_copy(
                    out=vhx_own[:, il // 2, il % 2, :, 0:HD],
                    in_=pv[:].rearrange("p a (x d) -> p (a x) d", d=HD))
            nc.scalar.dma_start(out=kag_v_in[:], in_=vhx_own[:])
            nc.gpsimd.collective_compute(
                "AllGather", mybir.AluOpType.bypass,
                ins=[kag_v_in[:]], outs=[kag_v_out[:]], replica_groups=RG8)
            # late-phase weights go behind the collectives on the gpsimd queue
            Wo_sb = pers.tile([P, CP, 2, C], LOWP)
            nc.gpsimd.dma_start(out=Wo_sb[:], in_=Wo_d[:])
            W1_sb = pers.tile([P, CP, 2, FF], FFNP)
            nc.gpsimd.dma_start(out=W1_sb[:], in_=W1_d[:])
            W2_sb = pers.tile([P, FF // P // 2, 2, C], FFNP)
            nc.gpsimd.dma_start(out=W2_sb[:], in_=W2_d[:])
            b2_b = pers.tile([P, C], BF16)
            nc.gpsimd.dma_start(out=b2_b[:], in_=b2_d[:])

            # Q-proj (own slice)
            for mh in range(CT // 2):
                pq = proj_pair(Wq_sb, qT_sb, mh, f"pq{mh}")
                for half in range(2):
                    m = 2 * mh + half
                    nc.scalar.activation(out=qhT3[:, m, :], in_=pq[:, half, :],
                                         func=AF.Identity, bias=bq_sb[:, m:m + 1])
            # residual init: x = q + (bo + bv@Wo)
            for j in range(JT):
                qraw = work.tile([P, C], BF16, tag="qraw", bufs=3, name=f"qraw{j}")
                nc.sync.dma_start(out=qraw[:], in_=qres_d[j * P:(j + 1) * P, :])
                nc.vector.tensor_add(out=x_acc[:, j, :], in0=qraw[:], in1=bres_b[:])
            # extract the group's four slices (own one included: it rewrites
            # identical bytes, keeping the SPMD program core-agnostic)
            pid = nc.sync.partition_id()
            kbase = (pid >> 2) * (4 * C)
            vbase = (pid >> 2) * (4 * P)
            for r in range(4):
                nc.sync.dma_start(
                    out=khT3[:, :, r * NT:(r + 1) * NT],
                    in_=kag_k_out[bass.ds(kbase + r * C, C), :]
                    .rearrange("(m p) f -> p m f", p=P))
                nc.sync.dma_start(
                    out=vhx[:, 2 * r:2 * r + 2, :, :, :],
                    in_=kag_v_out[bass.ds(vbase + r * P, P)])

            # ======= PHASE B: attention (out-proj deferred to PSUM) =======
            for pair in range(H // 2):
                for parity in range(2):
                    h = 2 * pair + parity
                    p0 = parity * HD
                    ctx_ps = ps.tile([HD + 1, NT], F32, tag="ctx", bufs=2,
                                     name=f"ctx{pair}_{parity}")
                    for mega in range(8):
                        s_ps = ps.tile([P, 2, NT], F32, tag="s", bufs=3,
                                       name=f"s{pair}_{mega}_{parity}")
                        for jj in range(2):
                            i = mega * 2 + jj
                            nc.tensor.matmul(
                                s_ps[:, jj, :],
                                khT3[p0:p0 + HD, pair, i * P:(i + 1) * P],
                                qhT3[p0:p0 + HD, pair, :],
                                start=True, stop=True)
                        expS = work.tile([P, 2, NT], LOWP, tag="expS", bufs=4,
                                         name=f"expS{pair}_{mega}_{parity}")
                        if mega >= 8 - N_DVE_EXP:
                            # Schraudolph exp2 on DVE: int bits of the target
                            # dtype approximate 2^x; the constant-offset error
                            # cancels in the softmax normalization.
                            nc.vector.tensor_scalar(
                                out=expS[:].bitcast(EXP_IDT), in0=s_ps[:],
                                scalar1=EXP_A, scalar2=EXP_B,
                                op0=ALU.mult, op1=ALU.add)
                        else:
                            nc.scalar.activation(out=expS[:], in_=s_ps[:],
                                                 func=AF.Exp, scale=ESCALE)
                        for jj in range(2):
                            i = mega * 2 + jj
                            if DR and jj == 0:
                                nc.tensor.matmul(
                                    ctx_ps[:], vhx[:, mega, :, h, :],
                                    expS[:], perf_mode=DR,
                                    start=(mega == 0), stop=(mega == 7))
                            elif not DR:
                                nc.tensor.matmul(
                                    ctx_ps[:], vhx[:, mega, jj, h, :],
                                    expS[:, jj, :],
                                    start=(mega == 0 and jj == 0),
                                    stop=(mega == 7 and jj == 1))
                    # normalize rows 0..63 by row 64 (broadcast via DRAM bounce)
                    rc_sb = work.tile([HD + 1, NT], BF16, tag="rc", bufs=2,
                                      name=f"rc{pair}_{parity}")
                    with nc.allow_low_precision(
                            reason="1/sumexp broadcast; 0.4% on a tiny term"):
                        nc.vector.reciprocal(out=rc_sb[HD:HD + 1, :],
                                             in_=ctx_ps[HD:HD + 1, :])
                    rrow = dsmall.tile([NT], BF16, tag="rrow",
                                       name=f"rrow{pair}_{parity}")
                    nc.sync.dma_start(out=rrow[:].rearrange("(o c) -> o c", o=1),
                                      in_=rc_sb[HD:HD + 1, :])
                    bc = work.tile([HD, NT], BF16, tag="bc", bufs=2,
                                   name=f"bc{pair}_{parity}")
                    nc.sync.dma_start(out=bc[:], in_=rrow[:].partition_broadcast(HD))
                    nc.vector.tensor_mul(
                        out=stack[p0:p0 + HD, pair // 2, pair % 2, :],
                        in0=ctx_ps[0:HD, :], in1=bc[:])

            # ======= deferred out-projection, LayerNorm interleaved =======
            hT3 = big.tile([P, CP, 2, NT], FFNP, tag="TB")
            mvs = work.tile([P, JT, 2], F32, tag="mvs", bufs=1)
            rstds = work.tile([P, JT], F32, tag="rstds", bufs=1)
            for j in range(JT):
                op = ps.tile([P, 2, NT], F32, tag="s", bufs=3, name=f"op{j}")
                for cp in range(CP):
                    for n in range(2):
                        if DR:
                            nc.tensor.matmul(
                                op[:, n, :], stack[:, cp, :, j * P:(j + 1) * P],
                                Wo_sb[:, cp, :, n * NT:(n + 1) * NT], perf_mode=DR,
                                start=(cp == 0), stop=(cp == CP - 1))
                        else:
                            for s in range(2):
                                nc.tensor.matmul(
                                    op[:, n, :], stack[:, cp, s, j * P:(j + 1) * P],
                                    Wo_sb[:, cp, s, n * NT:(n + 1) * NT],
                                    start=(cp == 0 and s == 0),
                                    stop=(cp == CP - 1 and s == 1))
                nc.vector.scalar_tensor_tensor(
                    out=x_acc[:, j, :],
                    in0=op[:].rearrange("p a b -> p (a b)"),
                    scalar=1.0 / (WS * WS), in1=x_acc[:, j, :],
                    op0=ALU.mult, op1=ALU.add)
                # LayerNorm for this token tile (ln_w/ln_b folded into W1/b1)
                st = work.tile([P, 2, 6], F32, tag="st", bufs=2, name=f"st{j}")
                for s in range(2):
                    nc.vector.bn_stats(out=st[:, s, :],
                                       in_=x_acc[:, j, s * NT:(s + 1) * NT])
                nc.vector.bn_aggr(out=mvs[:, j, :], in_=st[:])
                nc.scalar.activation(out=rstds[:, j:j + 1], in_=mvs[:, j, 1:2],
                                     func=AF.Sqrt, bias=eps_sb[:])
                nc.vector.reciprocal(out=rstds[:, j:j + 1], in_=rstds[:, j:j + 1])
                hj = work.tile([P, C], BF16, tag="hj", bufs=2, name=f"hj{j}")
                nc.vector.tensor_scalar(out=hj[:], in0=x_acc[:, j, :],
                                        scalar1=mvs[:, j, 0:1],
                                        scalar2=rstds[:, j:j + 1],
                                        op0=ALU.subtract, op1=ALU.mult)
                for k in range(2):
                    tp = ps.tile([P, CP, P], BF16, tag="ctx", bufs=2,
                                 name=f"tp{j}_{k}")
                    for t in range(CP):
                        nc.tensor.transpose(tp[:, t, :],
                                            hj[:, (k * CP + t) * P:(k * CP + t + 1) * P],
                                            ident[:])
                    nc.vector.tensor_copy(
                        out=hT3[:, k * 2:(k + 1) * 2, :, j * P:(j + 1) * P]
                        .rearrange("p a b f -> p (a b) f"),
                        in_=tp[:])

            # ======= FFN =======
            gT3 = big.tile([P, FF // P // 2, 2, NT], FFNP, tag="TA")
            for mh in range(FF // P // 2):
                pf = ps.tile([P, 2, NT], F32, tag="s", bufs=3, name=f"pf{mh}")
                for cp in range(CP):
                    for half in range(2):
                        mf = 2 * mh + half
                        if FFN_DR:
                            nc.tensor.matmul(pf[:, half, :],
                                             W1_sb[:, cp, :, mf * P:(mf + 1) * P],
                                             hT3[:, cp, :, :], perf_mode=FFN_DR,
                                             start=(cp == 0), stop=(cp == CP - 1))
                        else:
                            for s in range(2):
                                nc.tensor.matmul(
                                    pf[:, half, :],
                                    W1_sb[:, cp, s, mf * P:(mf + 1) * P],
                                    hT3[:, cp, s, :],
                                    start=(cp == 0 and s == 0),
                                    stop=(cp == CP - 1 and s == 1))
                for half in range(2):
                    mf = 2 * mh + half
                    nc.scalar.activation(out=gT3[:, mf // 2, mf % 2, :],
                                         in_=pf[:, half, :],
                                         func=AF.Gelu, bias=b1_sb[:, mf:mf + 1],
                                         scale=1.0 / WS)

            # FFN2: bf16 path streams the full 1024-wide W2 row per ff-tile
            for j in range(JT):
                xb = work.tile([P, C], F32, tag="xb", bufs=1, name=f"xb{j}")
                nc.vector.tensor_add(out=xb[:], in0=x_acc[:, j, :], in1=b2_b[:])
                out_sb = work.tile([P, C], F32, tag="outsb", bufs=2,
                                   name=f"osb{j}")
                pf2 = ps.tile([P, 2, NT], F32, tag="s", bufs=3, name=f"pf2{j}")
                for fp in range(FF // P // 2):
                    if FFN_DR:
                        for n in range(2):
                            nc.tensor.matmul(pf2[:, n, :],
                                             gT3[:, fp, :, j * P:(j + 1) * P],
                                             W2_sb[:, fp, :, n * NT:(n + 1) * NT],
                                             perf_mode=FFN_DR,
                                             start=(fp == 0), stop=(fp == 7))
                    else:
                        for s in range(2):
                            for n in range(2):
                                nc.tensor.matmul(
                                    pf2[:, n, :],
                                    gT3[:, fp, s, j * P:(j + 1) * P],
                                    W2_sb[:, fp, s, n * NT:(n + 1) * NT],
                                    start=(fp == 0 and s == 0),
                                    stop=(fp == 7 and s == 1))
                nc.vector.scalar_tensor_tensor(
                    out=out_sb[:], in0=pf2[:].rearrange("p a b -> p (a b)"),
                    scalar=1.0 / WS, in1=xb[:],
                    op0=ALU.mult, op1=ALU.add)
                nc.sync.dma_start(out=out_d[j * P:(j + 1) * P, :], in_=out_sb[:])

    nc.compile()
    return nc


_NC = None
LAST_RESULT = None


def kernel(q, k, v, Wq, bq, Wk, bk, Wv, bv, Wo, bo, ln_w, ln_b, W1, b1, W2, b2):
    global _NC, LAST_RESULT
    if _NC is None:
        _NC = build()
    lp = ml_dtypes.float8_e4m3 if FP8 else ml_dtypes.bfloat16
    fp = ml_dtypes.float8_e4m3 if FP8_FFN else ml_dtypes.bfloat16
    bf = ml_dtypes.bfloat16

    def wlay(w, dt, ff=C):
        # [C_in, F] -> [P, CP_in, 2, F] with c_in = (2cp+s)*128+p
        w = np.asarray(w, dtype=dt)
        ci = w.shape[0]
        return np.ascontiguousarray(
            w.reshape(ci // 256, 2, P, ff).transpose(2, 0, 1, 3))

    def xlay(x, dt):
        # [T, C] input slice -> transposed chunk-major [T//512, P, CP, 2, 512]
        xT = np.asarray(x, np.float32).T.astype(dt)      # [C, T]
        t = xT.shape[1]
        return np.ascontiguousarray(
            xT.reshape(CP, 2, P, t // NT, NT).transpose(3, 2, 0, 1, 4))

    Wq32, Wk32, Wv32, Wo32 = (np.asarray(w, np.float32) for w in (Wq, Wk, Wv, Wo))
    W132, W232 = np.asarray(W1, np.float32), np.asarray(W2, np.float32)
    lnw32, lnb32 = np.asarray(ln_w, np.float32), np.asarray(ln_b, np.float32)
    bres = (np.asarray(bo, np.float32) + np.asarray(bv, np.float32) @ Wo32)
    W1f = W132 * lnw32[:, None]
    b1f = np.asarray(b1, np.float32) + lnb32 @ W132

    shared = {
        "Wq": wlay(Wq32 * WS, lp), "Wk": wlay(Wk32 * WS, lp),
        "Wv": wlay(Wv32 * WS, lp), "Wo": wlay(Wo32 * WS, lp),
        "W1": wlay(W1f * WS, fp, ff=FF), "W2": wlay(W232 * WS, fp, ff=C),
        "bqt": np.ascontiguousarray(
            WS * np.asarray(bq, np.float32).reshape(CT, P).T),
        "bkt": np.ascontiguousarray(
            WS * np.asarray(bk, np.float32).reshape(CT, P).T),
        "b1t": np.ascontiguousarray(b1f.reshape(FF // P, P).T),
        "bresb": np.ascontiguousarray(np.broadcast_to(bres.astype(bf), (P, C))),
        "b2b": np.ascontiguousarray(
            np.broadcast_to(np.asarray(b2, bf), (P, C))),
    }
    in_maps = []
    for i in range(8):
        b, r = i // 4, i % 4
        m = dict(shared)
        m["kT"] = np.ascontiguousarray(
            xlay(k[b, r * NT:(r + 1) * NT], lp)[0])
        m["vT"] = np.ascontiguousarray(
            xlay(v[b, r * NT:(r + 1) * NT], lp)[0])
        m["qT"] = np.ascontiguousarray(
            xlay(q[b, r * NT:(r + 1) * NT], lp)[0])
        m["qres"] = np.ascontiguousarray(
            np.asarray(q[b, r * NT:(r + 1) * NT], np.float32).astype(bf))
        in_maps.append(m)
    LAST_RESULT = run_bass_kernel_spmd(_NC, in_maps, core_ids=list(range(8)))
    out = np.empty((B, N, C), np.float32)
    for i in range(8):
        b, r = i // 4, i % 4
        out[b, r * NT:(r + 1) * NT] = LAST_RESULT.results[i]["out"]
    return out
